# revision 1
# baseline (speedup 1.0000x reference)
"""Trainium2 Bass kernel for AdaptiveGraphConvolution (gnn_message_passing).

  pre_sup = x @ W                      [N, 64]
  s       = pre_sup[row] @ fw1 + pre_sup[col] @ fw2 + f_b     (per edge)
  deg     = bincount(row) + 1
  vals    = (deg[row] * deg[col]) ** (-s)
  out     = relu(segment_sum(vals[:, None] * pre_sup[col], row, N))

Strategy (8 NeuronCores, SPMD), core c owns destination nodes
[c*12500, (c+1)*12500):

  * Host does index-only preprocessing: edges bucketed by
    (dest-half, col-owner, 128-node dest window), padded to a cross-core
    uniform number of 128-edge tiles per bucket, sorted by col inside each
    bucket; indices localized (int16 owner-local col, window-local row).
  * Device phase A: pre_sup + per-node scalars via TensorE from x @ W_aug
    (W_aug carries W | W@fw2 | W@fw1); builds a 256B/row bf16 node table
    [vj(64) | l | b | 1 | b*l | 0...] and per-node row-side vector
    [a+fb | l | (a+fb)l | 1]; AllGather of the node table.
  * Phase B per (half, owner): one big SWDGE dma_gather (int16, owner-local)
    fetches per-edge source rows; per 128-edge tile the full 128x128
    (slot, node) weight matrix t = u(node)^T v(slot) comes from ONE rank-4
    TensorE matmul, ACT computes exp(-t), DVE masks by the one-hot of the
    edge's true dest row, and a second TensorE matmul (lhsT=masked weights,
    rhs=vj) accumulates the segment sum in a persistent PSUM block
    [128, 49 windows, 64]; ReLU + store at the end of the half.
"""

import sys

for _p in ("/opt/trn_rl_repo", "/opt/pypackages"):
    if _p not in sys.path:
        sys.path.append(_p)

import numpy as np
import ml_dtypes

import concourse.bass as bass
import concourse.bacc as bacc
import concourse.mybir as mybir
import concourse.tile as tile
from concourse.bass_utils import run_bass_kernel_spmd
from concourse.masks import make_identity

BF16 = ml_dtypes.bfloat16
P = 128
N_CORES = 8
ROWB = 128          # table row: 128 bf16 = 256B
CHUNK = 4           # tiles per exp/mask batch
N_SPLIT = 1         # gathers per (sweep, owner) block (SWDGE FIFO capacity)


# ----------------------------------------------------------------- host prep

def host_prep(row, col, n_nodes, n_cores):
    npc = n_nodes // n_cores
    npad = ((npc + P - 1) // P) * P
    n_win = npad // P
    n_sweep = 7 if n_win % 7 == 0 else (4 if n_win % 4 == 0 else 2)
    assert n_win % n_sweep == 0
    hw = n_win // n_sweep                            # windows per sweep

    row = np.asarray(row).astype(np.int64)
    col = np.asarray(col).astype(np.int64)
    order = np.argsort(row, kind="stable")
    row_s = row[order].astype(np.int32)
    col_s = col[order].astype(np.int32)

    percore = []
    cnt = np.zeros((n_cores, n_cores, n_win), np.int64)   # [core, owner, window]
    for c in range(n_cores):
        base = c * npc
        lo = np.searchsorted(row_s, base, "left")
        hi = np.searchsorted(row_s, base + npc, "left")
        r = (row_s[lo:hi] - base).astype(np.int32)
        cc = col_s[lo:hi]
        o = (cc // npc).astype(np.int32)
        w = r // P
        np.add.at(cnt[c], (o, w), 1)
        percore.append((r, cc, o, w))
    # uniform tiles per (owner, window) bucket across cores
    B = np.maximum((cnt.max(axis=0) + P - 1) // P, 1)     # [owner, window]
    n_tiles = int(B.sum())
    n_slots = n_tiles * P
    # slot offset of bucket (o, w): layout [half][owner][window][tiles]
    tile_base = np.zeros((n_cores, n_win), np.int64)
    t0 = 0
    order_buckets = []
    for hh in range(n_sweep):
        for oo in range(n_cores):
            for wi in range(hh * hw, (hh + 1) * hw):
                tile_base[oo, wi] = t0
                order_buckets.append((oo, wi))
                t0 += int(B[oo, wi])
    assert t0 == n_tiles

    shards = []
    for c in range(n_cores):
        r, cc, o, w = percore[c]
        idx16 = np.zeros(n_slots, np.int16)
        rloc = np.full(n_slots, -1.0, np.float32)
        bo = np.lexsort((cc, w, o))                  # sort by (owner, window, col)
        r, cc, o, w = r[bo], cc[bo], o[bo], w[bo]
        # start offset of each bucket's edges in the sorted list
        key = o.astype(np.int64) * n_win + w
        starts = np.searchsorted(key, np.arange(n_cores * n_win))
        ends = np.searchsorted(key, np.arange(n_cores * n_win), "right")
        for oo in range(n_cores):
            for wi in range(n_win):
                a, b = int(starts[oo * n_win + wi]), int(ends[oo * n_win + wi])
                if a == b:
                    continue
                s0 = int(tile_base[oo, wi]) * P
                k = b - a
                idx16[s0:s0 + k] = (cc[a:b] % npc).astype(np.int16)
                rloc[s0:s0 + k] = (r[a:b] - wi * P).astype(np.float32)
        deg = (np.bincount(r, minlength=npad) + 1).astype(np.float32)
        shards.append(dict(
            idx16=np.tile(np.ascontiguousarray(
                idx16.reshape(n_slots // 16, 16).T), (8, 1)),   # [128, S/16]
            rloc=np.ascontiguousarray(rloc.reshape(n_tiles, P).T),  # [P, n_tiles]
            deg=deg.reshape(npad, 1),
        ))
    L = dict(npc=npc, npad=npad, n_win=n_win, hw=hw, n_sweep=n_sweep, B=B,
             tile_base=tile_base, n_tiles=n_tiles)
    return shards, L


# ------------------------------------------------------------- device program

def build_program(L, in_dim, out_dim, n_cores):
    npad, n_win, hw = L["npad"], L["n_win"], L["hw"]
    n_sweep = L["n_sweep"]
    B, tile_base, n_tiles = L["B"], L["tile_base"], L["n_tiles"]
    n_k = in_dim // P
    f32, bf16, i16 = mybir.dt.float32, mybir.dt.bfloat16, mybir.dt.int16

    nc = bacc.Bacc("TRN2", target_bir_lowering=False, debug=False,
                   num_devices=n_cores)

    xin = nc.declare_dram_parameter("xin", [npad, in_dim], f32, isOutput=False)
    degp = nc.declare_dram_parameter("deg", [npad, 1], f32, isOutput=False)
    wmat = nc.declare_dram_parameter("wmat", [in_dim, out_dim], f32, isOutput=False)
    fw12 = nc.declare_dram_parameter("fw12", [out_dim, 2], f32, isOutput=False)
    fbrep = nc.declare_dram_parameter("fbrep", [P, 1], f32, isOutput=False)
    idxp = nc.declare_dram_parameter("idx16", [P, n_tiles * P // 16], i16,
                                     isOutput=False)
    rlocp = nc.declare_dram_parameter("rloc", [P, n_tiles], f32, isOutput=False)
    outp = nc.declare_dram_parameter("out", [npad, out_dim], f32, isOutput=True)

    with tile.TileContext(nc) as tc:
        with (
            tc.tile_pool(name="dram", bufs=1, space="DRAM") as dpool,
            tc.tile_pool(name="const", bufs=1) as cpool,
        ):
            t_loc = dpool.tile([npad, ROWB], bf16)
            t2_loc = dpool.tile([npad, 4], bf16)
            t_glob = dpool.tile([n_cores * npad, ROWB], bf16)

            identity = cpool.tile([P, P], bf16)
            make_identity(nc, identity[:])
            iota_i = cpool.tile([P, P], mybir.dt.int32)
            nc.gpsimd.iota(iota_i[:], pattern=[[1, P]], channel_multiplier=0)
            iota_bf = cpool.tile([P, P], bf16)
            nc.vector.tensor_copy(iota_bf[:], iota_i[:])
            fb_sb = cpool.tile([P, 1], f32)
            nc.sync.dma_start(out=fb_sb[:], in_=fbrep[:, :])
            fw_sb = cpool.tile([out_dim, 2], f32)
            nc.sync.dma_start(out=fw_sb[:], in_=fw12[:, :])
            fw_bf = cpool.tile([out_dim, 2], bf16)
            nc.vector.tensor_copy(fw_bf[:], fw_sb[:])

            # W_aug = [W | W@fw2 | W@fw1]  bf16 [P, n_k, 66]
            w_aug = cpool.tile([P, n_k, 66], bf16)
            with (
                tc.tile_pool(name="wtmp", bufs=2) as wpool,
                tc.tile_pool(name="wps", bufs=2, space="PSUM") as wps,
            ):
                wf = wpool.tile([P, n_k, out_dim], f32)
                nc.sync.dma_start(
                    out=wf[:], in_=wmat[:, :].rearrange("(s p) f -> p s f", p=P))
                nc.vector.tensor_copy(w_aug[:, :, 0:out_dim], wf[:])
                for s in range(n_k):
                    pT = wps.tile([out_dim, P], bf16, space="PSUM", tag="pT")
                    nc.tensor.transpose(out=pT[:], in_=w_aug[:, s, 0:out_dim],
                                        identity=identity[:])
                    wT = wpool.tile([out_dim, P], bf16, tag="wT")
                    nc.vector.tensor_copy(wT[:], pT[:])
                    pab = wps.tile([P, 2], f32, space="PSUM", tag="pab")
                    nc.tensor.matmul(out=pab[:], lhsT=wT[:], rhs=fw_bf[:],
                                     start=True, stop=True)
                    nc.vector.tensor_copy(w_aug[:, s, 64:65], pab[:, 1:2])  # b
                    nc.vector.tensor_copy(w_aug[:, s, 65:66], pab[:, 0:1])  # a

            # phase A: T row [vj(64)|l|b|1|bl|0...]; T2 row [a'|l|a'l|1]
            with (
                tc.tile_pool(name="xa", bufs=3) as xa,
                tc.tile_pool(name="psa", bufs=2, space="PSUM") as psa,
            ):
                for i in range(n_win):
                    xf = xa.tile([P, in_dim], f32, tag="xf")
                    nc.sync.dma_start(out=xf[:], in_=xin[i * P:(i + 1) * P, :])
                    xb = xa.tile([P, in_dim], bf16, tag="xb")
                    nc.vector.tensor_copy(xb[:], xf[:])
                    pt = psa.tile([P, 66], f32, space="PSUM", tag="pt")
                    for s in range(n_k):
                        pxt = psa.tile([P, P], bf16, space="PSUM", tag="pxt")
                        nc.tensor.transpose(out=pxt[:], in_=xb[:, s * P:(s + 1) * P],
                                            identity=identity[:])
                        xT = xa.tile([P, P], bf16, tag="xT")
                        nc.scalar.copy(xT[:], pxt[:])
                        nc.tensor.matmul(out=pt[:], lhsT=xT[:], rhs=w_aug[:, s, :],
                                         start=(s == 0), stop=(s == n_k - 1))
                    dg = xa.tile([P, 1], f32, tag="dg")
                    nc.sync.dma_start(out=dg[:], in_=degp[i * P:(i + 1) * P, :])
                    ldg = xa.tile([P, 1], f32, tag="ldg")
                    nc.scalar.activation(ldg[:], dg[:],
                                         mybir.ActivationFunctionType.Ln)
                    bl = xa.tile([P, 1], f32, tag="bl")
                    nc.vector.tensor_mul(bl[:], pt[:, 64:65], ldg[:])
                    tt = xa.tile([P, ROWB], bf16, tag="tt")
                    nc.vector.memset(tt[:, 68:ROWB], 0.0)
                    nc.vector.tensor_copy(tt[:, 0:64], pt[:, 0:64])
                    nc.vector.tensor_copy(tt[:, 64:65], ldg[:])
                    nc.vector.tensor_copy(tt[:, 65:66], pt[:, 64:65])
                    nc.vector.memset(tt[:, 66:67], 1.0)
                    nc.vector.tensor_copy(tt[:, 67:68], bl[:])
                    ap_ = xa.tile([P, 1], f32, tag="ap_")
                    nc.vector.tensor_add(ap_[:], pt[:, 65:66], fb_sb[:])
                    al = xa.tile([P, 1], f32, tag="al")
                    nc.vector.tensor_mul(al[:], ap_[:], ldg[:])
                    t2t = xa.tile([P, 4], bf16, tag="t2t")
                    nc.vector.tensor_copy(t2t[:, 0:1], ap_[:])
                    nc.vector.tensor_copy(t2t[:, 1:2], ldg[:])
                    nc.vector.tensor_copy(t2t[:, 2:3], al[:])
                    nc.vector.memset(t2t[:, 3:4], 1.0)
                    nc.sync.dma_start(out=t_loc[i * P:(i + 1) * P, :], in_=tt[:])
                    nc.sync.dma_start(out=t2_loc[i * P:(i + 1) * P, :], in_=t2t[:])

            nc.gpsimd.collective_compute(
                "AllGather", mybir.AluOpType.bypass,
                replica_groups=[list(range(n_cores))],
                ins=[t_loc.opt()], outs=[t_glob.opt()],
            )

            # ---------------- phase B
            dma_sem = nc.alloc_semaphore("dg_dma")
            prep_sem = nc.alloc_semaphore("dg_prep")
            n_gather = 0

            with (
                tc.tile_pool(name="ub", bufs=1) as ub,
                tc.tile_pool(name="gb", bufs=3) as gb,
                tc.tile_pool(name="wb", bufs=3) as wbp,
                tc.tile_pool(name="psb", bufs=1, space="PSUM") as psb,
                tc.tile_pool(name="psc", bufs=2, space="PSUM") as psc,
                tc.tile_pool(name="psd", bufs=4, space="PSUM") as psd,
            ):
                for hh in range(n_sweep):
                    u_sb = ub.tile([4, hw * P], bf16, tag="u_sb")
                    for wi in range(hw):
                        gwin = hh * hw + wi
                        t2w = wbp.tile([P, 4], bf16, tag="t2w")
                        nc.sync.dma_start(
                            out=t2w[:], in_=t2_loc[gwin * P:(gwin + 1) * P, :])
                        put = psd.tile([CHUNK * 4, P], bf16, space="PSUM", tag="ptr")
                        nc.tensor.transpose(out=put[0:4, :], in_=t2w[:],
                                            identity=identity[:])
                        nc.scalar.copy(u_sb[:, wi * P:(wi + 1) * P], put[0:4, :])

                    po = psb.tile([P, hw, out_dim], f32, space="PSUM", tag="po")
                    nc.vector.memset(po[:], 0.0)

                    def wins_of(tid0_, nt_):
                        out = []
                        for oo_ in range(n_cores):
                            t_lo_ = int(tile_base[oo_, hh * hw])
                            for wi_ in range(hw):
                                for _b in range(int(B[oo_, hh * hw + wi_])):
                                    out.append(wi_)
                        # global (within sweep) tile index -> window
                        base_ = int(tile_base[0, hh * hw])
                        return out[tid0_ - base_:tid0_ - base_ + nt_]

                    def process_tiles(tg, nt, tid0, wins, rl):
                        for ch0 in range(0, nt, CHUNK):
                            m = min(CHUNK, nt - ch0)
                            pt_ = psc.tile([P, CHUNK, P], f32, space="PSUM",
                                           tag="pt_")
                            v_tiles = []
                            for q in range(m):
                                pvt = psd.tile([4, P], bf16,
                                               space="PSUM", tag="ptr")
                                nc.tensor.transpose(
                                    out=pvt[:],
                                    in_=tg[:, ch0 + q, 64:68],
                                    identity=identity[:])
                                v_q = wbp.tile([4, P], bf16, tag="v_sb")
                                if (ch0 + q) % 2 == 1:
                                    nc.scalar.copy(v_q[:], pvt[:])
                                else:
                                    nc.vector.tensor_copy(v_q[:], pvt[:])
                                v_tiles.append(v_q)
                            for q in range(m):
                                wq = wins[ch0 + q]
                                nc.tensor.matmul(
                                    out=pt_[:, q, :],
                                    lhsT=v_tiles[q][:],
                                    rhs=u_sb[:, wq * P:(wq + 1) * P],
                                    start=True, stop=True)
                            ex = wbp.tile([P, CHUNK, P], bf16, tag="ex")
                            nc.scalar.activation(
                                ex[:, 0:m, :], pt_[:, 0:m, :],
                                mybir.ActivationFunctionType.Exp, scale=-1.0)
                            # batched one-hot: (rloc bcast) == iota  [P, m, P]
                            mk = wbp.tile([P, CHUNK, P], bf16, tag="mk")
                            nc.vector.tensor_tensor(
                                out=mk[:, 0:m, :],
                                in0=rl[:, ch0:ch0 + m].to_broadcast([P, m, P]),
                                in1=iota_bf[:, None, :].to_broadcast([P, m, P]),
                                op=mybir.AluOpType.is_equal)
                            msk = wbp.tile([P, CHUNK, P], bf16, tag="msk")
                            nc.vector.tensor_mul(
                                msk[:, 0:m, :], mk[:, 0:m, :], ex[:, 0:m, :])
                            for q in range(m):
                                wq = wins[ch0 + q]
                                nc.tensor.matmul(
                                    out=po[:, wq, :],
                                    lhsT=msk[:, q, :],
                                    rhs=tg[:, ch0 + q, 0:64],
                                    start=False, stop=False,
                                    skip_group_check=True)

                    process_queue = []
                    pending_tg = None
                    blocks = []
                    for oo in range(n_cores):
                        t_lo = int(tile_base[oo, hh * hw])
                        nt_all = int(B[oo, hh * hw:(hh + 1) * hw].sum())
                        splits = np.array_split(np.arange(nt_all), N_SPLIT)
                        for sp in splits:
                            if len(sp):
                                blocks.append((oo, t_lo + int(sp[0]), len(sp)))
                    for oo, tid0, nt in blocks:
                        if True:
                            n_idx = nt * P
                            tg = gb.tile([P, nt, ROWB], bf16, tag="tg")
                            ixs = gb.tile([P, n_idx // 16], i16, tag="ixs")
                            c0 = tid0 * P // 16
                            nc.sync.dma_start(
                                out=ixs[:], in_=idxp[:, c0:c0 + n_idx // 16])
                            with tc.tile_critical():
                                nc.gpsimd.dma_gather(
                                    out_ap=tg[:],
                                    in_ap=t_glob[oo * npad:(oo + 1) * npad, :],
                                    idxs_ap=ixs[:],
                                    num_idxs=n_idx, num_idxs_reg=n_idx,
                                    elem_size=ROWB, single_packet=False,
                                    prepare_only=True, sem=dma_sem,
                                ).then_inc(prep_sem, 1)
                                n_gather += 1
                                nc.gpsimd.wait_ge(prep_sem, n_gather)
                                nc.gpsimd.trigger_dma(count=1)
                                if pending_tg is not None:
                                    nc.gpsimd.wait_ge(dma_sem, 16 * (n_gather - 1))
                                    nc.gpsimd.tensor_copy(
                                        pending_tg[:, :, 63:64],
                                        pending_tg[:, :, 63:64])
                                    nc.gpsimd.tensor_copy(
                                        pending_tg[:, :, 64:68],
                                        pending_tg[:, :, 64:68])
                            process_queue.append(
                                (tg, nt, tid0, wins_of(tid0, nt)))
                            pending_tg = tg
                            if len(process_queue) < 2:
                                continue
                            tg, nt, tid0, wins = process_queue.pop(0)

                            rl = gb.tile([P, nt], f32, tag="rl")
                            nc.sync.dma_start(
                                out=rl[:], in_=rlocp[:, tid0:tid0 + nt])
                            process_tiles(tg, nt, tid0, wins, rl)

                    # drain remaining gathers
                    for tg, nt, tid0, wins in process_queue:
                        with tc.tile_critical():
                            nc.gpsimd.wait_ge(dma_sem, 16 * n_gather)
                            nc.gpsimd.tensor_copy(
                                tg[:, :, 63:64], tg[:, :, 63:64])
                            nc.gpsimd.tensor_copy(
                                tg[:, :, 64:68], tg[:, :, 64:68])
                        rl = gb.tile([P, nt], f32, tag="rl")
                        nc.sync.dma_start(
                            out=rl[:], in_=rlocp[:, tid0:tid0 + nt])
                        process_tiles(tg, nt, tid0, wins, rl)
                    process_queue = []
                    pending_tg = None

                    for wi in range(hw):
                        gwin = hh * hw + wi
                        ob = wbp.tile([P, out_dim], f32, tag="ob")
                        nc.scalar.activation(ob[:], po[:, wi, :],
                                             mybir.ActivationFunctionType.Relu)
                        nc.sync.dma_start(
                            out=outp[gwin * P:(gwin + 1) * P, :], in_=ob[:])

    nc.compile()
    return nc


# ------------------------------------------------------------------ assemble

def make_in_maps(x, W_, f_w, f_b, shards, L, n_cores):
    npc, npad, in_dim = L["npc"], L["npad"], x.shape[1]
    fw12 = np.stack([f_w[:64, 0], f_w[64:, 0]], axis=1).astype(np.float32)
    fbrep = np.full((P, 1), np.float32(f_b[0]), np.float32)
    in_maps = []
    for c in range(n_cores):
        xsh = np.zeros((npad, in_dim), np.float32)
        xsh[:npc] = x[c * npc:(c + 1) * npc]
        in_maps.append({
            "xin": xsh,
            "deg": shards[c]["deg"],
            "wmat": np.ascontiguousarray(W_, np.float32),
            "fw12": fw12,
            "fbrep": fbrep,
            "idx16": shards[c]["idx16"],
            "rloc": shards[c]["rloc"],
        })
    return in_maps


def kernel(x, W, f_w, f_b, row, col, _profile=None):
    x = np.asarray(x, np.float32)
    W = np.asarray(W, np.float32)
    f_w = np.asarray(f_w, np.float32)
    f_b = np.asarray(f_b, np.float32)
    n = x.shape[0]

    shards, L = host_prep(row, col, n, N_CORES)
    nc = build_program(L, x.shape[1], 64, N_CORES)
    in_maps = make_in_maps(x, W, f_w, f_b, shards, L, N_CORES)
    res = run_bass_kernel_spmd(
        nc, in_maps, core_ids=list(range(N_CORES)), trace=_profile is not None)
    if _profile is not None and isinstance(_profile, dict):
        _profile["exec_time_ns"] = res.exec_time_ns
        _profile["mean_exec_time_ns"] = res.mean_exec_time_ns

    npc = L["npc"]
    out = np.empty((n, 64), np.float32)
    for c in range(N_CORES):
        out[c * npc:(c + 1) * npc] = res.results[c]["out"][:npc]
    return out



# revision 3
# speedup vs baseline: 1.0641x; 1.0641x over previous
"""Trainium2 Bass kernel for AdaptiveGraphConvolution (gnn_message_passing).

  pre_sup = x @ W                      [N, 64]
  s       = pre_sup[row] @ fw1 + pre_sup[col] @ fw2 + f_b     (per edge)
  deg     = bincount(row) + 1
  vals    = (deg[row] * deg[col]) ** (-s)
  out     = relu(segment_sum(vals[:, None] * pre_sup[col], row, N))

Strategy (8 NeuronCores, SPMD), core c owns destination nodes
[c*12500, (c+1)*12500):

  * Host does index-only preprocessing: edges bucketed by
    (dest-half, col-owner, 128-node dest window), padded to a cross-core
    uniform number of 128-edge tiles per bucket, sorted by col inside each
    bucket; indices localized (int16 owner-local col, window-local row).
  * Device phase A: pre_sup + per-node scalars via TensorE from x @ W_aug
    (W_aug carries W | W@fw2 | W@fw1); builds a 256B/row bf16 node table
    [vj(64) | l | b | 1 | b*l | 0...] and per-node row-side vector
    [a+fb | l | (a+fb)l | 1]; AllGather of the node table.
  * Phase B per (half, owner): one big SWDGE dma_gather (int16, owner-local)
    fetches per-edge source rows; per 128-edge tile the full 128x128
    (slot, node) weight matrix t = u(node)^T v(slot) comes from ONE rank-4
    TensorE matmul, ACT computes exp(-t), DVE masks by the one-hot of the
    edge's true dest row, and a second TensorE matmul (lhsT=masked weights,
    rhs=vj) accumulates the segment sum in a persistent PSUM block
    [128, 49 windows, 64]; ReLU + store at the end of the half.
"""

import sys

for _p in ("/opt/trn_rl_repo", "/opt/pypackages"):
    if _p not in sys.path:
        sys.path.append(_p)

import numpy as np
import ml_dtypes

import concourse.bass as bass
import concourse.bacc as bacc
import concourse.mybir as mybir
import concourse.tile as tile
from concourse.bass_utils import run_bass_kernel_spmd
from concourse.masks import make_identity

BF16 = ml_dtypes.bfloat16
P = 128
N_CORES = 8
ROWB = 128          # table row: 128 bf16 = 256B
CHUNK = 4           # tiles per exp/mask batch
N_SPLIT = 1         # gathers per (sweep, owner) block (SWDGE FIFO capacity)


# ----------------------------------------------------------------- host prep

def host_prep(row, col, n_nodes, n_cores):
    npc = n_nodes // n_cores
    npad = ((npc + P - 1) // P) * P
    n_win = npad // P
    n_sweep = 7 if n_win % 7 == 0 else (4 if n_win % 4 == 0 else 2)
    assert n_win % n_sweep == 0
    hw = n_win // n_sweep                            # windows per sweep

    row = np.asarray(row).astype(np.int64)
    col = np.asarray(col).astype(np.int64)
    order = np.argsort(row, kind="stable")
    row_s = row[order].astype(np.int32)
    col_s = col[order].astype(np.int32)

    percore = []
    cnt = np.zeros((n_cores, n_cores, n_win), np.int64)   # [core, owner, window]
    for c in range(n_cores):
        base = c * npc
        lo = np.searchsorted(row_s, base, "left")
        hi = np.searchsorted(row_s, base + npc, "left")
        r = (row_s[lo:hi] - base).astype(np.int32)
        cc = col_s[lo:hi]
        o = (cc // npc).astype(np.int32)
        w = r // P
        np.add.at(cnt[c], (o, w), 1)
        percore.append((r, cc, o, w))
    # uniform tiles per (owner, window) bucket across cores
    B = np.maximum((cnt.max(axis=0) + P - 1) // P, 1)     # [owner, window]
    n_tiles = int(B.sum())
    n_slots = n_tiles * P
    # slot offset of bucket (o, w): layout [half][owner][window][tiles]
    tile_base = np.zeros((n_cores, n_win), np.int64)
    t0 = 0
    order_buckets = []
    for hh in range(n_sweep):
        for oo in range(n_cores):
            for wi in range(hh * hw, (hh + 1) * hw):
                tile_base[oo, wi] = t0
                order_buckets.append((oo, wi))
                t0 += int(B[oo, wi])
    assert t0 == n_tiles

    shards = []
    for c in range(n_cores):
        r, cc, o, w = percore[c]
        idx16 = np.zeros(n_slots, np.int16)
        rloc = np.full(n_slots, -1.0, np.float32)
        bo = np.lexsort((cc, w, o))                  # sort by (owner, window, col)
        r, cc, o, w = r[bo], cc[bo], o[bo], w[bo]
        # start offset of each bucket's edges in the sorted list
        key = o.astype(np.int64) * n_win + w
        starts = np.searchsorted(key, np.arange(n_cores * n_win))
        ends = np.searchsorted(key, np.arange(n_cores * n_win), "right")
        for oo in range(n_cores):
            for wi in range(n_win):
                a, b = int(starts[oo * n_win + wi]), int(ends[oo * n_win + wi])
                if a == b:
                    continue
                s0 = int(tile_base[oo, wi]) * P
                k = b - a
                idx16[s0:s0 + k] = (cc[a:b] % npc).astype(np.int16)
                rloc[s0:s0 + k] = (r[a:b] - wi * P).astype(np.float32)
        deg = (np.bincount(r, minlength=npad) + 1).astype(np.float32)
        shards.append(dict(
            idx16=np.tile(np.ascontiguousarray(
                idx16.reshape(n_slots // 16, 16).T), (8, 1)),   # [128, S/16]
            rloc=np.ascontiguousarray(rloc.reshape(n_tiles, P).T),  # [P, n_tiles]
            deg=deg.reshape(npad, 1),
        ))
    L = dict(npc=npc, npad=npad, n_win=n_win, hw=hw, n_sweep=n_sweep, B=B,
             tile_base=tile_base, n_tiles=n_tiles)
    return shards, L


# ------------------------------------------------------------- device program

def build_program(L, in_dim, out_dim, n_cores):
    npad, n_win, hw = L["npad"], L["n_win"], L["hw"]
    n_sweep = L["n_sweep"]
    B, tile_base, n_tiles = L["B"], L["tile_base"], L["n_tiles"]
    n_k = in_dim // P
    f32, bf16, i16 = mybir.dt.float32, mybir.dt.bfloat16, mybir.dt.int16

    nc = bacc.Bacc("TRN2", target_bir_lowering=False, debug=False,
                   num_devices=n_cores)

    xin = nc.declare_dram_parameter("xin", [npad, in_dim], f32, isOutput=False)
    degp = nc.declare_dram_parameter("deg", [npad, 1], f32, isOutput=False)
    wmat = nc.declare_dram_parameter("wmat", [in_dim, out_dim], f32, isOutput=False)
    fw12 = nc.declare_dram_parameter("fw12", [out_dim, 2], f32, isOutput=False)
    fbrep = nc.declare_dram_parameter("fbrep", [P, 1], f32, isOutput=False)
    idxp = nc.declare_dram_parameter("idx16", [P, n_tiles * P // 16], i16,
                                     isOutput=False)
    rlocp = nc.declare_dram_parameter("rloc", [P, n_tiles], f32, isOutput=False)
    outp = nc.declare_dram_parameter("out", [npad, out_dim], f32, isOutput=True)

    with tile.TileContext(nc) as tc:
        with (
            tc.tile_pool(name="dram", bufs=1, space="DRAM") as dpool,
            tc.tile_pool(name="const", bufs=1) as cpool,
        ):
            t_loc = dpool.tile([npad, ROWB], bf16)
            t2_loc = dpool.tile([npad, 4], bf16)
            t_glob = dpool.tile([n_cores * npad, ROWB], bf16)

            identity = cpool.tile([P, P], bf16)
            make_identity(nc, identity[:])
            iota_i = cpool.tile([P, P], mybir.dt.int32)
            nc.gpsimd.iota(iota_i[:], pattern=[[1, P]], channel_multiplier=0)
            iota_bf = cpool.tile([P, P], bf16)
            nc.vector.tensor_copy(iota_bf[:], iota_i[:])
            fb_sb = cpool.tile([P, 1], f32)
            nc.sync.dma_start(out=fb_sb[:], in_=fbrep[:, :])
            fw_sb = cpool.tile([out_dim, 2], f32)
            nc.sync.dma_start(out=fw_sb[:], in_=fw12[:, :])
            fw_bf = cpool.tile([out_dim, 2], bf16)
            nc.vector.tensor_copy(fw_bf[:], fw_sb[:])

            # W_aug = [W | W@fw2 | W@fw1]  bf16 [P, n_k, 66]
            w_aug = cpool.tile([P, n_k, 66], bf16)
            with (
                tc.tile_pool(name="wtmp", bufs=2) as wpool,
                tc.tile_pool(name="wps", bufs=2, space="PSUM") as wps,
            ):
                wf = wpool.tile([P, n_k, out_dim], f32)
                nc.sync.dma_start(
                    out=wf[:], in_=wmat[:, :].rearrange("(s p) f -> p s f", p=P))
                nc.vector.tensor_copy(w_aug[:, :, 0:out_dim], wf[:])
                for s in range(n_k):
                    pT = wps.tile([out_dim, P], bf16, space="PSUM", tag="pT")
                    nc.tensor.transpose(out=pT[:], in_=w_aug[:, s, 0:out_dim],
                                        identity=identity[:])
                    wT = wpool.tile([out_dim, P], bf16, tag="wT")
                    nc.vector.tensor_copy(wT[:], pT[:])
                    pab = wps.tile([P, 2], f32, space="PSUM", tag="pab")
                    nc.tensor.matmul(out=pab[:], lhsT=wT[:], rhs=fw_bf[:],
                                     start=True, stop=True)
                    nc.vector.tensor_copy(w_aug[:, s, 64:65], pab[:, 1:2])  # b
                    nc.vector.tensor_copy(w_aug[:, s, 65:66], pab[:, 0:1])  # a

            # phase A: T row [vj(64)|l|b|1|bl|0...]; T2 row [a'|l|a'l|1]
            with (
                tc.tile_pool(name="xa", bufs=3) as xa,
                tc.tile_pool(name="psa", bufs=2, space="PSUM") as psa,
            ):
                for i in range(n_win):
                    xf = xa.tile([P, in_dim], f32, tag="xf")
                    nc.sync.dma_start(out=xf[:], in_=xin[i * P:(i + 1) * P, :])
                    xb = xa.tile([P, in_dim], bf16, tag="xb")
                    nc.vector.tensor_copy(xb[:], xf[:])
                    pt = psa.tile([P, 66], f32, space="PSUM", tag="pt")
                    for s in range(n_k):
                        pxt = psa.tile([P, P], bf16, space="PSUM", tag="pxt")
                        nc.tensor.transpose(out=pxt[:], in_=xb[:, s * P:(s + 1) * P],
                                            identity=identity[:])
                        xT = xa.tile([P, P], bf16, tag="xT")
                        nc.scalar.copy(xT[:], pxt[:])
                        nc.tensor.matmul(out=pt[:], lhsT=xT[:], rhs=w_aug[:, s, :],
                                         start=(s == 0), stop=(s == n_k - 1))
                    dg = xa.tile([P, 1], f32, tag="dg")
                    nc.sync.dma_start(out=dg[:], in_=degp[i * P:(i + 1) * P, :])
                    ldg = xa.tile([P, 1], f32, tag="ldg")
                    nc.scalar.activation(ldg[:], dg[:],
                                         mybir.ActivationFunctionType.Ln)
                    bl = xa.tile([P, 1], f32, tag="bl")
                    nc.vector.tensor_mul(bl[:], pt[:, 64:65], ldg[:])
                    tt = xa.tile([P, ROWB], bf16, tag="tt")
                    nc.vector.memset(tt[:, 68:ROWB], 0.0)
                    nc.vector.tensor_copy(tt[:, 0:64], pt[:, 0:64])
                    nc.vector.tensor_copy(tt[:, 64:65], ldg[:])
                    nc.vector.tensor_copy(tt[:, 65:66], pt[:, 64:65])
                    nc.vector.memset(tt[:, 66:67], 1.0)
                    nc.vector.tensor_copy(tt[:, 67:68], bl[:])
                    ap_ = xa.tile([P, 1], f32, tag="ap_")
                    nc.vector.tensor_add(ap_[:], pt[:, 65:66], fb_sb[:])
                    al = xa.tile([P, 1], f32, tag="al")
                    nc.vector.tensor_mul(al[:], ap_[:], ldg[:])
                    t2t = xa.tile([P, 4], bf16, tag="t2t")
                    nc.vector.tensor_copy(t2t[:, 0:1], ap_[:])
                    nc.vector.tensor_copy(t2t[:, 1:2], ldg[:])
                    nc.vector.tensor_copy(t2t[:, 2:3], al[:])
                    nc.vector.memset(t2t[:, 3:4], 1.0)
                    nc.sync.dma_start(out=t_loc[i * P:(i + 1) * P, :], in_=tt[:])
                    nc.sync.dma_start(out=t2_loc[i * P:(i + 1) * P, :], in_=t2t[:])

            nc.gpsimd.collective_compute(
                "AllGather", mybir.AluOpType.bypass,
                replica_groups=[list(range(n_cores))],
                ins=[t_loc.opt()], outs=[t_glob.opt()],
            )

            # ---------------- phase B
            dma_sem = nc.alloc_semaphore("dg_dma")
            prep_sem = nc.alloc_semaphore("dg_prep")
            n_gather = 0

            with (
                tc.tile_pool(name="ub", bufs=1) as ub,
                tc.tile_pool(name="gb", bufs=3) as gb,
                tc.tile_pool(name="wb", bufs=3) as wbp,
                tc.tile_pool(name="psb", bufs=1, space="PSUM") as psb,
                tc.tile_pool(name="psc", bufs=2, space="PSUM") as psc,
                tc.tile_pool(name="psd", bufs=4, space="PSUM") as psd,
            ):
                for hh in range(n_sweep):
                    u_sb = ub.tile([4, hw * P], bf16, tag="u_sb")
                    for wi in range(hw):
                        gwin = hh * hw + wi
                        t2w = wbp.tile([P, 4], bf16, tag="t2w")
                        nc.sync.dma_start(
                            out=t2w[:], in_=t2_loc[gwin * P:(gwin + 1) * P, :])
                        put = psd.tile([CHUNK * 4, P], bf16, space="PSUM", tag="ptr")
                        nc.tensor.transpose(out=put[0:4, :], in_=t2w[:],
                                            identity=identity[:])
                        nc.scalar.copy(u_sb[:, wi * P:(wi + 1) * P], put[0:4, :])

                    po = psb.tile([P, hw, out_dim], f32, space="PSUM", tag="po")
                    nc.vector.memset(po[:], 0.0)

                    def wins_of(tid0_, nt_):
                        out = []
                        for oo_ in range(n_cores):
                            t_lo_ = int(tile_base[oo_, hh * hw])
                            for wi_ in range(hw):
                                for _b in range(int(B[oo_, hh * hw + wi_])):
                                    out.append(wi_)
                        # global (within sweep) tile index -> window
                        base_ = int(tile_base[0, hh * hw])
                        return out[tid0_ - base_:tid0_ - base_ + nt_]

                    def process_tiles(tg, nt, tid0, wins, rl):
                        for ch0 in range(0, nt, CHUNK):
                            m = min(CHUNK, nt - ch0)
                            pt_ = psc.tile([P, CHUNK, P], f32, space="PSUM",
                                           tag="pt_")
                            v_tiles = []
                            for q in range(m):
                                pvt = psd.tile([4, P], bf16,
                                               space="PSUM", tag="ptr")
                                nc.tensor.transpose(
                                    out=pvt[:],
                                    in_=tg[:, ch0 + q, 64:68],
                                    identity=identity[:])
                                v_q = wbp.tile([4, P], bf16, tag="v_sb")
                                if (ch0 + q) % 2 == 1:
                                    nc.scalar.copy(v_q[:], pvt[:])
                                else:
                                    nc.vector.tensor_copy(v_q[:], pvt[:])
                                v_tiles.append(v_q)
                            for q in range(m):
                                wq = wins[ch0 + q]
                                nc.tensor.matmul(
                                    out=pt_[:, q, :],
                                    lhsT=v_tiles[q][:],
                                    rhs=u_sb[:, wq * P:(wq + 1) * P],
                                    start=True, stop=True)
                            ex = wbp.tile([P, CHUNK, P], bf16, tag="ex")
                            nc.scalar.activation(
                                ex[:, 0:m, :], pt_[:, 0:m, :],
                                mybir.ActivationFunctionType.Exp, scale=-1.0)
                            # batched one-hot: (rloc bcast) == iota  [P, m, P]
                            mk = wbp.tile([P, CHUNK, P], bf16, tag="mk")
                            nc.vector.tensor_tensor(
                                out=mk[:, 0:m, :],
                                in0=rl[:, ch0:ch0 + m].to_broadcast([P, m, P]),
                                in1=iota_bf[:, None, :].to_broadcast([P, m, P]),
                                op=mybir.AluOpType.is_equal)
                            msk = wbp.tile([P, CHUNK, P], bf16, tag="msk")
                            nc.vector.tensor_mul(
                                msk[:, 0:m, :], mk[:, 0:m, :], ex[:, 0:m, :])
                            for q in range(m):
                                wq = wins[ch0 + q]
                                nc.tensor.matmul(
                                    out=po[:, wq, :],
                                    lhsT=msk[:, q, :],
                                    rhs=tg[:, ch0 + q, 0:64],
                                    start=False, stop=False,
                                    skip_group_check=True)

                    process_queue = []
                    pending_tg = None
                    blocks = []
                    for oo in range(n_cores):
                        t_lo = int(tile_base[oo, hh * hw])
                        nt_all = int(B[oo, hh * hw:(hh + 1) * hw].sum())
                        splits = np.array_split(np.arange(nt_all), N_SPLIT)
                        for sp in splits:
                            if len(sp):
                                blocks.append((oo, t_lo + int(sp[0]), len(sp)))
                    for oo, tid0, nt in blocks:
                        if True:
                            n_idx = nt * P
                            tg = gb.tile([P, nt, ROWB], bf16, tag="tg")
                            ixs = gb.tile([P, n_idx // 16], i16, tag="ixs")
                            c0 = tid0 * P // 16
                            nc.sync.dma_start(
                                out=ixs[:], in_=idxp[:, c0:c0 + n_idx // 16])
                            with tc.tile_critical(no_gpsimd_drain=True):
                                nc.gpsimd.dma_gather(
                                    out_ap=tg[:],
                                    in_ap=t_glob[oo * npad:(oo + 1) * npad, :],
                                    idxs_ap=ixs[:],
                                    num_idxs=n_idx, num_idxs_reg=n_idx,
                                    elem_size=ROWB, single_packet=False,
                                    prepare_only=True, sem=dma_sem,
                                ).then_inc(prep_sem, 1)
                                n_gather += 1
                                nc.gpsimd.wait_ge(prep_sem, n_gather)
                                nc.gpsimd.trigger_dma(count=1)
                                if pending_tg is not None:
                                    nc.vector.wait_ge(dma_sem, 16 * (n_gather - 1))
                                    nc.vector.tensor_copy(
                                        pending_tg[:, :, 63:64],
                                        pending_tg[:, :, 63:64])
                                    nc.vector.tensor_copy(
                                        pending_tg[:, :, 64:68],
                                        pending_tg[:, :, 64:68])
                            process_queue.append(
                                (tg, nt, tid0, wins_of(tid0, nt)))
                            pending_tg = tg
                            if len(process_queue) < 2:
                                continue
                            tg, nt, tid0, wins = process_queue.pop(0)

                            rl = gb.tile([P, nt], f32, tag="rl")
                            nc.sync.dma_start(
                                out=rl[:], in_=rlocp[:, tid0:tid0 + nt])
                            process_tiles(tg, nt, tid0, wins, rl)

                    # drain remaining gathers
                    for tg, nt, tid0, wins in process_queue:
                        with tc.tile_critical(no_gpsimd_drain=True):
                            nc.vector.wait_ge(dma_sem, 16 * n_gather)
                            nc.vector.tensor_copy(
                                tg[:, :, 63:64], tg[:, :, 63:64])
                            nc.vector.tensor_copy(
                                tg[:, :, 64:68], tg[:, :, 64:68])
                        rl = gb.tile([P, nt], f32, tag="rl")
                        nc.sync.dma_start(
                            out=rl[:], in_=rlocp[:, tid0:tid0 + nt])
                        process_tiles(tg, nt, tid0, wins, rl)
                    process_queue = []
                    pending_tg = None

                    for wi in range(hw):
                        gwin = hh * hw + wi
                        ob = wbp.tile([P, out_dim], f32, tag="ob")
                        nc.scalar.activation(ob[:], po[:, wi, :],
                                             mybir.ActivationFunctionType.Relu)
                        nc.sync.dma_start(
                            out=outp[gwin * P:(gwin + 1) * P, :], in_=ob[:])

    nc.compile()
    return nc


# ------------------------------------------------------------------ assemble

def make_in_maps(x, W_, f_w, f_b, shards, L, n_cores):
    npc, npad, in_dim = L["npc"], L["npad"], x.shape[1]
    fw12 = np.stack([f_w[:64, 0], f_w[64:, 0]], axis=1).astype(np.float32)
    fbrep = np.full((P, 1), np.float32(f_b[0]), np.float32)
    in_maps = []
    for c in range(n_cores):
        xsh = np.zeros((npad, in_dim), np.float32)
        xsh[:npc] = x[c * npc:(c + 1) * npc]
        in_maps.append({
            "xin": xsh,
            "deg": shards[c]["deg"],
            "wmat": np.ascontiguousarray(W_, np.float32),
            "fw12": fw12,
            "fbrep": fbrep,
            "idx16": shards[c]["idx16"],
            "rloc": shards[c]["rloc"],
        })
    return in_maps


def kernel(x, W, f_w, f_b, row, col, _profile=None):
    x = np.asarray(x, np.float32)
    W = np.asarray(W, np.float32)
    f_w = np.asarray(f_w, np.float32)
    f_b = np.asarray(f_b, np.float32)
    n = x.shape[0]

    shards, L = host_prep(row, col, n, N_CORES)
    nc = build_program(L, x.shape[1], 64, N_CORES)
    in_maps = make_in_maps(x, W, f_w, f_b, shards, L, N_CORES)
    res = run_bass_kernel_spmd(
        nc, in_maps, core_ids=list(range(N_CORES)), trace=_profile is not None)
    if _profile is not None and isinstance(_profile, dict):
        _profile["exec_time_ns"] = res.exec_time_ns
        _profile["mean_exec_time_ns"] = res.mean_exec_time_ns

    npc = L["npc"]
    out = np.empty((n, 64), np.float32)
    for c in range(N_CORES):
        out[c * npc:(c + 1) * npc] = res.results[c]["out"][:npc]
    return out

